# revision 33
# baseline (speedup 1.0000x reference)
"""Trainium2 Bass kernel for nn_AttentionModule (v5).

Computation (per batch row b, input feature i):
    E      = tanh(x @ E_W + E_b)                      # [B, 50]
    s      = einsum('be,iea->bia', E, A_W) + A_b      # [B, 66, 20]
    A      = softmax(s, -1)[..., 1]                   # [B, 66]
    out    = x * A

Math rewrite: softmax(s)[1] = 1 / (1 + sum_{a!=1} exp(s_a - s_1)).
Weights are pre-differenced vs column a=1 on the host (that column
becomes identically zero and is dropped -> 19 kept columns); the bias is
folded into the matmul via a constant-1 row of E (tanh(30) == 1.0).

v5 changes vs v4 (engine re-balance; ACT was 88% busy, window 5.5us/macro):
  - Per-i, the 7 softmax columns with the SMALLEST average softmax mass
    (estimated on the host from an 8k-row sample) are computed with a
    16-bit Schraudolph exp on the POOL engine: the matmul emits
    t = 128*(127 - c + s*log2e) (scale+bias folded into W2 on the host);
    one fp32->int16 converting tensor_copy writes the bit pattern of
    bf16 ~= exp(s).  Mass-sorting keeps total L2 err ~2e-3 (gate 2e-2).
  - The remaining 12 exact columns exp on ACT (bf16 out).
  - Fold tree: uniform bf16 2x-packed tensor_tensor adds on DVE at
    2-macro granularity; the "+1" rides the final scalar_tensor_tensor.
  - reciprocal_approx_fast on DVE (2-macro), final x*rec on Pool,
    y store per 2-macro group.
Engine budget/macro (cost model): ACT 3.99us, DVE 3.71us, Pool 3.52us,
PE 2.35us, vs v4 ACT 4.89us.

Pure data-parallel over 8 cores (32768 rows/core; 64 macros of 512 rows,
grouped in 32 pairs == DMA groups).
"""

import numpy as np

B_TOTAL, INPUT, E_NODE, A_NODE = 262144, 66, 50, 20
N_CORES = 8
B_LOCAL = B_TOTAL // N_CORES          # 32768
NBLK = 4                              # 128-row blocks per macro
MACRO = 128 * NBLK                    # 512
DM = 2                                # macros per group (pair)
GROUP = DM * MACRO                    # 1024 rows
PBLK = DM * NBLK                      # 8 blocks per pair
CONST_ROW_BIAS = 30.0                 # tanh(30) == 1.0 in fp32

A_RED = A_NODE - 1                    # 19 kept softmax columns
N_APPROX = 5                          # columns exp'd via DVE Schraudolph
N_EXACT = A_RED - N_APPROX            # 14 columns exp'd on ACT
NIA = INPUT * A_RED                   # 1254
IPC = INPUT // 2                      # 33 i-groups per exact chunk
CH_EX = IPC * N_EXACT                 # 462 exact cols per chunk (2 chunks)
CH_EXP = CH_EX                        # already even
CH_AP = INPUT * N_APPROX              # 264 approx cols (single chunk)
EX_W = 14                             # dense exp tile (odd-offset slices)
IX_W = 5                              # dense ix tile
NG = INPUT * NBLK                     # 264 groups per macro
NGP = INPUT * PBLK                    # 528 groups per pair

SCHRAUD_A = 128.0 / float(np.log(2))            # int16 variant
SCHRAUD_C = 0.0440                              # mantissa shift (tunable)

DMA_MACROS = DM                       # kept for test.py --small sizing

BUFS = {"xtp": 3, "xp": 3, "etp": 3, "expp": 2, "ixp": 2, "dnp": 4,
        "outp": 2}

_CACHE = {}


def _build_bass(n_rows, repeat=1):
    import concourse.bass as bass
    import concourse.bacc as bacc
    import concourse.tile as tile
    from concourse import mybir
    from contextlib import ExitStack

    f32 = mybir.dt.float32
    f32r = mybir.dt.float32r
    bf16 = mybir.dt.bfloat16
    i16 = mybir.dt.int16
    n_groups = n_rows // GROUP
    assert n_rows % GROUP == 0

    nc = bacc.Bacc("TRN2", target_bir_lowering=False, debug=False,
                   num_devices=N_CORES)

    x_d = nc.dram_tensor("x", [n_rows, INPUT], f32, kind="ExternalInput").ap()
    xt_d = nc.dram_tensor("xT", [INPUT, n_rows], f32r,
                          kind="ExternalInput").ap()
    w1_d = nc.dram_tensor("W1", [INPUT, E_NODE + 1], f32r,
                          kind="ExternalInput").ap()
    b1_d = nc.dram_tensor("b1", [E_NODE + 1, 1], f32,
                          kind="ExternalInput").ap()
    w2e_d = nc.dram_tensor("W2E", [E_NODE + 1, 2 * CH_EXP], f32r,
                           kind="ExternalInput").ap()
    w2a_d = nc.dram_tensor("W2A", [E_NODE + 1, CH_AP], f32r,
                           kind="ExternalInput").ap()
    y_d = nc.dram_tensor("y", [n_rows, INPUT], f32, kind="ExternalOutput").ap()

    x_r = x_d.rearrange("(m p) f -> m p f", p=128)
    y_r = y_d.rearrange("(m p) f -> m p f", p=128)

    add = mybir.AluOpType.add
    mult = mybir.AluOpType.mult

    with tile.TileContext(nc) as tc, ExitStack() as ctx:
        const = ctx.enter_context(tc.tile_pool(name="const", bufs=1))
        xtp = ctx.enter_context(tc.tile_pool(name="xtp", bufs=BUFS["xtp"]))
        xp = ctx.enter_context(tc.tile_pool(name="xp", bufs=BUFS["xp"]))
        etp = ctx.enter_context(tc.tile_pool(name="etp", bufs=BUFS["etp"]))
        expp = ctx.enter_context(tc.tile_pool(name="expp", bufs=BUFS["expp"]))
        ixp = ctx.enter_context(tc.tile_pool(name="ixp", bufs=BUFS["ixp"]))
        dnp = ctx.enter_context(tc.tile_pool(name="dnp", bufs=BUFS["dnp"]))
        outp = ctx.enter_context(tc.tile_pool(name="outp", bufs=BUFS["outp"]))
        ps_et = ctx.enter_context(tc.tile_pool(name="ps_et", bufs=1,
                                               space="PSUM"))
        ps_se = ctx.enter_context(tc.tile_pool(name="ps_se", bufs=2,
                                               space="PSUM"))
        ps_sa = ctx.enter_context(tc.tile_pool(name="ps_sa", bufs=2,
                                               space="PSUM"))

        w1_sb = const.tile([INPUT, E_NODE + 1], f32r)
        nc.sync.dma_start(out=w1_sb, in_=w1_d)
        b1_sb = const.tile([E_NODE + 1, 1], f32)
        nc.sync.dma_start(out=b1_sb, in_=b1_d)
        w2e_sb = const.tile([E_NODE + 1, 2 * CH_EXP], f32r)
        nc.sync.dma_start(out=w2e_sb, in_=w2e_d)
        w2a_sb = const.tile([E_NODE + 1, CH_AP], f32r)
        nc.sync.dma_start(out=w2a_sb, in_=w2a_d)

        iters = [g for _ in range(repeat) for g in range(n_groups)]

        def emit_loads(it):
            """DMA loads for one 1024-row pair (issued 2 groups ahead)."""
            g = iters[it]
            xgt = xtp.tile([INPUT, GROUP], f32r, name="xgt")
            nc.sync.dma_start(out=xgt, in_=xt_d[:, g * GROUP:(g + 1) * GROUP])
            xg = xp.tile([128, PBLK, INPUT], f32, name="xg")
            nc.sync.dma_start(
                out=xg,
                in_=x_r[g * PBLK:(g + 1) * PBLK].rearrange("m p f -> p m f"),
            )
            return xgt, xg

        def emit_front_pair(it):
            """mm1 + tanh for one PAIR: single 2-bank ET PSUM tile, one
            tanh instruction per 1024 rows (halves ACT instr overhead)."""
            xgt, xg = loads_of[it]
            et_ps = ps_et.tile([E_NODE + 1, 2, MACRO], f32)
            for h in range(2):
                nc.tensor.matmul(et_ps[:, h], w1_sb,
                                 xgt[:, h * MACRO:(h + 1) * MACRO],
                                 start=True, stop=True)
            et_sb = etp.tile([E_NODE + 1, GROUP], f32r)
            nc.scalar.activation(
                et_sb.rearrange("p (h w) -> p h w", h=2), et_ps,
                mybir.ActivationFunctionType.Tanh,
                bias=b1_sb, scale=1.0,
            )
            return et_sb

        def emit_pair(it, prev):
            """mm2 + exp + schraudolph-conv for the pair, with the previous
            pair's tail stages interleaved between blocks."""
            exp_pr = expp.tile([128, PBLK, INPUT, EX_W], bf16,
                               name="exp_pr")
            ix_pr = ixp.tile([128, PBLK, INPUT, IX_W], i16, name="ix_pr")
            tail = _tail_stages(prev) if prev is not None else []
            ti = 0
            for b in range(PBLK):
                et_sb = front_of[it]
                lhs = et_sb[:, b * 128:(b + 1) * 128]
                # separate exact / approx PSUM tiles: exp (ACT) and the
                # Schraudolph convert (Pool) must not share a tile, or the
                # tile tracker chains the two readers and the PSUM recycle
                # serializes the whole pipeline.
                # chunk stride 512: PSUM matmul outputs must be
                # bank-aligned
                s_ex = ps_se.tile([128, 2, 512], f32)
                with tc.high_priority():
                    for c in range(2):
                        nc.tensor.matmul(
                            s_ex[:, c, 0:CH_EXP], lhs,
                            w2e_sb[:, c * CH_EXP:(c + 1) * CH_EXP],
                            start=True, stop=True,
                        )
                nc.scalar.activation(
                    exp_pr[:, b, :, 0:N_EXACT]
                    .rearrange("p (c w) a -> p c w a", c=2),
                    s_ex[:, :, 0:CH_EX]
                    .rearrange("p c (w a) -> p c w a", a=N_EXACT),
                    mybir.ActivationFunctionType.Exp,
                )
                s_ap = ps_sa.tile([128, CH_AP], f32)
                nc.tensor.matmul(s_ap, lhs, w2a_sb, start=True, stop=True)
                # Schraudolph exp-16: matmul emitted t = 128*(127-c+s*log2e);
                # fp32->int16 convert writes the bit pattern of bf16~exp(s).
                # (Pool cannot read PSUM on TRN2, so this rides DVE.)
                with tc.high_priority():
                    nc.vector.tensor_copy(
                        out=ix_pr[:, b, :, 0:N_APPROX],
                        in_=s_ap[:, 0:CH_AP]
                        .rearrange("p (g a) -> p g a", a=N_APPROX),
                    )
                # interleave ~1-2 deferred tail stages per block
                want = (b + 1) * len(tail) // PBLK
                while ti < want:
                    tail[ti]()
                    ti += 1
            while ti < len(tail):
                tail[ti]()
                ti += 1
            return exp_pr, ix_pr, it

        def _tail_stages(state):
            """den/recip/multiply/store for a pair (deferred one pair)."""
            exp_pr, ix_pr, it = state
            EX = exp_pr.rearrange("p b g a -> p (b g) a")     # [128,528,12]
            AP = ix_pr.bitcast(bf16).rearrange("p b g a -> p (b g) a")
            dens = [dnp.tile([128, NG], f32, name=f"den{h}")
                    for h in range(2)]
            recs = [dnp.tile([128, NG], f32, name=f"rec{h}")
                    for h in range(2)]

            # Per-macro tail stages (h = macro half of the previous pair).
            # All packed slices start at even element offsets (2x_1p needs
            # 4B alignment on HW).  Chain per half:
            #   e0 -> e1 -> e2 -> x0 -> m(Pool) -> m1(Pool) -> r -> mul
            def sl(T, h, a0, a1):
                return T[:, h * NG:(h + 1) * NG, a0:a1]

            def e0(h):  # 14 -> 7   (7w packed, DVE; odd in1 offset)
                nc.vector.tensor_tensor(out=sl(EX, h, 0, 7),
                                        in0=sl(EX, h, 0, 7),
                                        in1=sl(EX, h, 7, 14), op=add)

            def e1(h):  # 7 -> 3 + [6]   (3w packed, DVE)
                nc.vector.tensor_tensor(out=sl(EX, h, 0, 3),
                                        in0=sl(EX, h, 0, 3),
                                        in1=sl(EX, h, 3, 6), op=add)

            def e2(h):  # ex0 += ex2   (1w, DVE)
                nc.vector.tensor_tensor(out=sl(EX, h, 0, 1),
                                        in0=sl(EX, h, 0, 1),
                                        in1=sl(EX, h, 2, 3), op=add)

            def p0(h):  # 5 -> 2 + [4]   (2w packed, DVE)
                nc.vector.tensor_tensor(out=sl(AP, h, 0, 2),
                                        in0=sl(AP, h, 0, 2),
                                        in1=sl(AP, h, 2, 4), op=add)

            def x0(h):  # ex[0:2] += ap[0:2]   (2w packed, DVE)
                nc.vector.tensor_tensor(out=sl(EX, h, 0, 2),
                                        in0=sl(EX, h, 0, 2),
                                        in1=sl(AP, h, 0, 2), op=add)

            def n0(h):  # ex1 += ex6; then ex1 += ap4 via n1   (1w, Pool)
                nc.gpsimd.tensor_tensor(out=sl(EX, h, 1, 2),
                                        in0=sl(EX, h, 1, 2),
                                        in1=sl(EX, h, 6, 7), op=add)

            def n1(h):  # ex1 += ap4   (1w, Pool)
                nc.gpsimd.tensor_tensor(out=sl(EX, h, 1, 2),
                                        in0=sl(EX, h, 1, 2),
                                        in1=sl(AP, h, 4, 5), op=add)

            def m(h):   # den = ex0 + ex1   (bf16 -> f32, Pool)
                nc.gpsimd.tensor_tensor(
                    out=dens[h].rearrange("p (g a) -> p g a", a=1),
                    in0=sl(EX, h, 0, 1), in1=sl(EX, h, 1, 2), op=add)

            def m1(h):  # den += 1   (the dropped a=1 column, Pool)
                nc.gpsimd.tensor_scalar_add(dens[h], dens[h], 1.0)

            def r(h):
                nc.vector.reciprocal_approx_fast(out=recs[h], in_=dens[h])

            def mul(h):
                _, xg = loads_of[it]
                og = outp.tile([128, NBLK, INPUT], f32, name="og")
                nc.gpsimd.tensor_tensor(
                    out=og, in0=xg[:, h * NBLK:(h + 1) * NBLK],
                    in1=recs[h].rearrange("p (t f) -> p t f", f=INPUT),
                    op=mult)
                g = iters[it]
                m0 = g * PBLK + h * NBLK
                nc.sync.dma_start(
                    out=y_r[m0:m0 + NBLK].rearrange("m p f -> p m f"),
                    in_=og,
                )

            from functools import partial
            out = []
            for h in range(2):
                for st in (e0, e1, e2, p0, x0, n0, n1, m, m1, r, mul):
                    out.append(partial(st, h))
            return out

        loads_of = {0: emit_loads(0)}
        if len(iters) > 1:
            loads_of[1] = emit_loads(1)
        front_of = {0: emit_front_pair(0)}
        pending = None
        for it in range(len(iters)):
            if it + 2 < len(iters):
                loads_of[it + 2] = emit_loads(it + 2)
            if it + 1 < len(iters):
                front_of[it + 1] = emit_front_pair(it + 1)
            pending = emit_pair(it, pending)
            front_of.pop(it, None)
            if it > 1:
                loads_of.pop(it - 2, None)
        for stage in _tail_stages(pending):
            stage()

    nc.compile()
    return nc


def _select_columns(x, E_W, E_b, A_W, A_b):
    """Per-i ranking of the 19 kept softmax columns by mean softmax mass
    (8k-row sample).  Returns [66, 19] array of kept-column indices,
    largest-mass first (first N_EXACT -> exact, rest -> approx)."""
    rng = np.random.default_rng(0)
    n = min(8192, x.shape[0])
    xs = x[:n].astype(np.float32)
    E = np.tanh(xs @ E_W + E_b)
    s = np.einsum("be,iea->bia", E, A_W.astype(np.float32)) + A_b
    sm = s - s.max(axis=2, keepdims=True)
    p = np.exp(sm)
    p /= p.sum(axis=2, keepdims=True)
    mass = p.mean(axis=0)                       # [66, 20]
    keep = np.array([a for a in range(A_NODE) if a != 1])
    mass19 = mass[:, keep]                      # [66, 19]
    order = np.argsort(-mass19, axis=1)         # descending mass
    return keep[order]                          # [66, 19] original indices


def _prep_weights(x, E_W, E_b, A_W, A_b):
    E_W = np.asarray(E_W, dtype=np.float32)
    E_b = np.asarray(E_b, dtype=np.float32)
    A_W = np.asarray(A_W, dtype=np.float32)
    A_b = np.asarray(A_b, dtype=np.float32)
    w1 = np.concatenate([E_W, np.zeros((INPUT, 1), np.float32)], axis=1)
    b1 = np.concatenate([E_b, np.float32([CONST_ROW_BIAS])]).reshape(-1, 1)
    dW = A_W - A_W[:, :, 1:2]                        # [66, 50, 20]
    db = A_b - A_b[:, 1:2]                           # [66, 20]
    cols = _select_columns(x, E_W, E_b, A_W, A_b)    # [66, 19]
    # reorder per-i: exact (large mass) first, then approx
    dW_s = np.take_along_axis(dW, cols[:, None, :], axis=2)   # [66, 50, 19]
    db_s = np.take_along_axis(db, cols, axis=1)               # [66, 19]
    w2f = np.concatenate(
        [dW_s.transpose(1, 0, 2),
         db_s.reshape(INPUT, 1, A_RED).transpose(1, 0, 2)], axis=0,
    ).astype(np.float64)                             # [51, 66, 19]
    # Schraudolph-16 folding for the last N_APPROX cols of each i:
    # t = s*128*log2(e) + 128*(127-c); bias rides the const-1 ET row.
    B16 = (127.0 - SCHRAUD_C) * 128.0
    w2f[:, :, N_EXACT:] *= SCHRAUD_A
    w2f[E_NODE, :, N_EXACT:] += B16
    # exact: 2 chunks of 33 i-groups; approx: one 264-col chunk (i-major)
    w2c = w2f.reshape(E_NODE + 1, 2, IPC, A_RED)
    w2e = np.zeros((E_NODE + 1, 2, CH_EXP), np.float64)
    w2e[:, :, :CH_EX] = w2c[:, :, :, :N_EXACT].reshape(E_NODE + 1, 2, CH_EX)
    w2e = w2e.reshape(E_NODE + 1, 2 * CH_EXP).astype(np.float32)
    w2a = np.ascontiguousarray(
        w2f[:, :, N_EXACT:].reshape(E_NODE + 1, CH_AP).astype(np.float32))
    return np.ascontiguousarray(w1), np.ascontiguousarray(b1), \
        np.ascontiguousarray(w2e), np.ascontiguousarray(w2a)


def _make_in_maps(x, E_W, E_b, A_W, A_b):
    x = np.ascontiguousarray(np.asarray(x, dtype=np.float32))
    n_local = x.shape[0] // N_CORES
    w1, b1, w2e, w2a = _prep_weights(x, E_W, E_b, A_W, A_b)
    in_maps = []
    for i in range(N_CORES):
        xi = x[i * n_local:(i + 1) * n_local]
        in_maps.append({
            "x": xi,
            "xT": np.ascontiguousarray(xi.T),
            "W1": w1, "b1": b1, "W2E": w2e, "W2A": w2a,
        })
    return in_maps, n_local


def _run(x, E_W, E_b, A_W, A_b, trace=False):
    from concourse.bass_utils import run_bass_kernel_spmd

    in_maps, n_local = _make_in_maps(x, E_W, E_b, A_W, A_b)
    key = ("nc", n_local)
    if key not in _CACHE:
        _CACHE[key] = _build_bass(n_local)
    nc = _CACHE[key]
    res = run_bass_kernel_spmd(nc, in_maps, list(range(N_CORES)), trace=trace)
    out = np.concatenate([res.results[i]["y"] for i in range(N_CORES)], axis=0)
    return out, res


def kernel(x, E_W, E_b, A_W, A_b):
    out, _ = _run(x, E_W, E_b, A_W, A_b, trace=False)
    return out


# revision 46
# speedup vs baseline: 4.0470x; 4.0470x over previous
"""Trainium2 Bass kernel for nn_AttentionModule (v5).

Computation (per batch row b, input feature i):
    E      = tanh(x @ E_W + E_b)                      # [B, 50]
    s      = einsum('be,iea->bia', E, A_W) + A_b      # [B, 66, 20]
    A      = softmax(s, -1)[..., 1]                   # [B, 66]
    out    = x * A

Math rewrite: softmax(s)[1] = 1 / (1 + sum_{a!=1} exp(s_a - s_1)).
Weights are pre-differenced vs column a=1 on the host (that column
becomes identically zero and is dropped -> 19 kept columns); the bias is
folded into the matmul via a constant-1 row of E (tanh(30) == 1.0).

v5 changes vs v4 (engine re-balance; ACT was 88% busy, window 5.5us/macro):
  - Per-i, the 7 softmax columns with the SMALLEST average softmax mass
    (estimated on the host from an 8k-row sample) are computed with a
    16-bit Schraudolph exp on the POOL engine: the matmul emits
    t = 128*(127 - c + s*log2e) (scale+bias folded into W2 on the host);
    one fp32->int16 converting tensor_copy writes the bit pattern of
    bf16 ~= exp(s).  Mass-sorting keeps total L2 err ~2e-3 (gate 2e-2).
  - The remaining 12 exact columns exp on ACT (bf16 out).
  - Fold tree: uniform bf16 2x-packed tensor_tensor adds on DVE at
    2-macro granularity; the "+1" rides the final scalar_tensor_tensor.
  - reciprocal_approx_fast on DVE (2-macro), final x*rec on Pool,
    y store per 2-macro group.
Engine budget/macro (cost model): ACT 3.99us, DVE 3.71us, Pool 3.52us,
PE 2.35us, vs v4 ACT 4.89us.

Pure data-parallel over 8 cores (32768 rows/core; 64 macros of 512 rows,
grouped in 32 pairs == DMA groups).
"""

import numpy as np

B_TOTAL, INPUT, E_NODE, A_NODE = 262144, 66, 50, 20
N_CORES = 8
B_LOCAL = B_TOTAL // N_CORES          # 32768
NBLK = 4                              # 128-row blocks per macro
MACRO = 128 * NBLK                    # 512
DM = 2                                # macros per group (pair)
GROUP = DM * MACRO                    # 1024 rows
PBLK = DM * NBLK                      # 8 blocks per pair
CONST_ROW_BIAS = 30.0                 # tanh(30) == 1.0 in fp32

A_RED = A_NODE - 1                    # 19 kept softmax columns
N_APPROX = 4                          # columns exp'd via DVE Schraudolph
N_EXACT = A_RED - N_APPROX            # 15 columns exp'd on ACT
NIA = INPUT * A_RED                   # 1254
IPC = INPUT // 2                      # 33 i-groups per exact chunk
CH_EX = IPC * N_EXACT                 # 495 exact cols per chunk (2 chunks)
CH_EXP = CH_EX + 1                    # matmul cols padded even (f32r needs
                                      # an even moving-free size)
CH_AP = INPUT * N_APPROX              # 264 approx cols (single chunk)
EX_W = 16                             # exp tile group width (4B alignment)
IX_W = 4                              # ix tile group width
NG = INPUT * NBLK                     # 264 groups per macro
NGP = INPUT * PBLK                    # 528 groups per pair

SCHRAUD_A = 128.0 / float(np.log(2))            # int16 variant
SCHRAUD_C = 0.0440                              # mantissa shift (tunable)

DMA_MACROS = DM                       # kept for test.py --small sizing

BUFS = {"xtp": 3, "xp": 3, "etp": 3, "expp": 2, "ixp": 2, "dnp": 4,
        "outp": 2}

_CACHE = {}


def _build_bass(n_rows, repeat=1):
    import concourse.bass as bass
    import concourse.bacc as bacc
    import concourse.tile as tile
    from concourse import mybir
    from contextlib import ExitStack

    f32 = mybir.dt.float32
    f32r = mybir.dt.float32r
    bf16 = mybir.dt.bfloat16
    i16 = mybir.dt.int16
    n_groups = n_rows // GROUP
    assert n_rows % GROUP == 0

    nc = bacc.Bacc("TRN2", target_bir_lowering=False, debug=False,
                   num_devices=N_CORES)

    x_d = nc.dram_tensor("x", [n_rows, INPUT], f32, kind="ExternalInput").ap()
    xt_d = nc.dram_tensor("xT", [INPUT, n_rows], f32r,
                          kind="ExternalInput").ap()
    w1_d = nc.dram_tensor("W1", [INPUT, E_NODE + 1], f32r,
                          kind="ExternalInput").ap()
    b1_d = nc.dram_tensor("b1", [E_NODE + 1, 1], f32,
                          kind="ExternalInput").ap()
    w2e_d = nc.dram_tensor("W2E", [E_NODE + 1, 2 * CH_EXP], f32r,
                           kind="ExternalInput").ap()
    w2a_d = nc.dram_tensor("W2A", [E_NODE + 1, CH_AP], f32r,
                           kind="ExternalInput").ap()
    y_d = nc.dram_tensor("y", [n_rows, INPUT], f32, kind="ExternalOutput").ap()

    x_r = x_d.rearrange("(m p) f -> m p f", p=128)
    y_r = y_d.rearrange("(m p) f -> m p f", p=128)

    add = mybir.AluOpType.add
    mult = mybir.AluOpType.mult

    with tile.TileContext(nc) as tc, ExitStack() as ctx:
        const = ctx.enter_context(tc.tile_pool(name="const", bufs=1))
        xtp = ctx.enter_context(tc.tile_pool(name="xtp", bufs=BUFS["xtp"]))
        xp = ctx.enter_context(tc.tile_pool(name="xp", bufs=BUFS["xp"]))
        etp = ctx.enter_context(tc.tile_pool(name="etp", bufs=BUFS["etp"]))
        expp = ctx.enter_context(tc.tile_pool(name="expp", bufs=BUFS["expp"]))
        ixp = ctx.enter_context(tc.tile_pool(name="ixp", bufs=BUFS["ixp"]))
        dnp = ctx.enter_context(tc.tile_pool(name="dnp", bufs=BUFS["dnp"]))
        outp = ctx.enter_context(tc.tile_pool(name="outp", bufs=BUFS["outp"]))
        ps_et = ctx.enter_context(tc.tile_pool(name="ps_et", bufs=1,
                                               space="PSUM"))
        ps_se = ctx.enter_context(tc.tile_pool(name="ps_se", bufs=2,
                                               space="PSUM"))
        ps_sa = ctx.enter_context(tc.tile_pool(name="ps_sa", bufs=2,
                                               space="PSUM"))

        w1_sb = const.tile([INPUT, E_NODE + 1], f32r)
        nc.sync.dma_start(out=w1_sb, in_=w1_d)
        b1_sb = const.tile([E_NODE + 1, 1], f32)
        nc.sync.dma_start(out=b1_sb, in_=b1_d)
        w2e_sb = const.tile([E_NODE + 1, 2 * CH_EXP], f32r)
        nc.sync.dma_start(out=w2e_sb, in_=w2e_d)
        w2a_sb = const.tile([E_NODE + 1, CH_AP], f32r)
        nc.sync.dma_start(out=w2a_sb, in_=w2a_d)

        iters = [g for _ in range(repeat) for g in range(n_groups)]

        def emit_loads(it):
            """DMA loads for one 1024-row pair (issued 2 groups ahead)."""
            g = iters[it]
            xgt = xtp.tile([INPUT, GROUP], f32r, name="xgt")
            nc.sync.dma_start(out=xgt, in_=xt_d[:, g * GROUP:(g + 1) * GROUP])
            xg = xp.tile([128, PBLK, INPUT], f32, name="xg")
            nc.sync.dma_start(
                out=xg,
                in_=x_r[g * PBLK:(g + 1) * PBLK].rearrange("m p f -> p m f"),
            )
            return xgt, xg

        def emit_front_pair(it):
            """mm1 + tanh for one PAIR: single 2-bank ET PSUM tile, one
            tanh instruction per 1024 rows (halves ACT instr overhead)."""
            xgt, xg = loads_of[it]
            et_ps = ps_et.tile([E_NODE + 1, 2, MACRO], f32)
            for h in range(2):
                nc.tensor.matmul(et_ps[:, h], w1_sb,
                                 xgt[:, h * MACRO:(h + 1) * MACRO],
                                 start=True, stop=True)
            et_sb = etp.tile([E_NODE + 1, GROUP], f32r)
            nc.scalar.activation(
                et_sb.rearrange("p (h w) -> p h w", h=2), et_ps,
                mybir.ActivationFunctionType.Tanh,
                bias=b1_sb, scale=1.0,
            )
            return et_sb

        def emit_pair(it, prev):
            """mm2 + exp + schraudolph-conv for the pair, with the previous
            pair's tail stages interleaved between blocks."""
            exp_pr = expp.tile([128, PBLK, INPUT, EX_W], bf16,
                               name="exp_pr")
            ix_pr = ixp.tile([128, PBLK, INPUT, IX_W], i16, name="ix_pr")
            tail = _tail_stages(prev) if prev is not None else []
            ti = 0
            for b in range(PBLK):
                if b == 5 and it + 1 < len(iters):
                    # emit the next pair's front mid-pair: the tanh then
                    # lands where ACT idles on the s_ex recycle anyway
                    front_of[it + 1] = emit_front_pair(it + 1)
                et_sb = front_of[it]
                lhs = et_sb[:, b * 128:(b + 1) * 128]
                # separate exact / approx PSUM tiles: exp (ACT) and the
                # Schraudolph convert (Pool) must not share a tile, or the
                # tile tracker chains the two readers and the PSUM recycle
                # serializes the whole pipeline.
                # chunk stride 512: PSUM matmul outputs must be
                # bank-aligned
                s_ex = ps_se.tile([128, 2, 512], f32)
                with tc.high_priority():
                    for c in range(2):
                        nc.tensor.matmul(
                            s_ex[:, c, 0:CH_EXP], lhs,
                            w2e_sb[:, c * CH_EXP:(c + 1) * CH_EXP],
                            start=True, stop=True,
                        )
                nc.scalar.activation(
                    exp_pr[:, b, :, 0:N_EXACT]
                    .rearrange("p (c w) a -> p c w a", c=2),
                    s_ex[:, :, 0:CH_EX]
                    .rearrange("p c (w a) -> p c w a", a=N_EXACT),
                    mybir.ActivationFunctionType.Exp,
                )
                s_ap = ps_sa.tile([128, CH_AP], f32)
                nc.tensor.matmul(s_ap, lhs, w2a_sb, start=True, stop=True)
                # Schraudolph exp-16: matmul emitted t = 128*(127-c+s*log2e);
                # fp32->int16 convert writes the bit pattern of bf16~exp(s).
                # (Pool cannot read PSUM on TRN2, so this rides DVE.)
                with tc.high_priority():
                    nc.vector.tensor_copy(
                        out=ix_pr[:, b, :, 0:N_APPROX],
                        in_=s_ap[:, 0:CH_AP]
                        .rearrange("p (g a) -> p g a", a=N_APPROX),
                    )
                # interleave ~1-2 deferred tail stages per block
                want = (b + 1) * len(tail) // PBLK
                while ti < want:
                    tail[ti]()
                    ti += 1
            while ti < len(tail):
                tail[ti]()
                ti += 1
            return exp_pr, ix_pr, it

        def _tail_stages(state, parts=2):
            """den/recip/multiply/store for a pair (deferred one pair).
            parts=2: per-macro stages; the final pair uses parts=4 so the
            drain chain at kernel end is half as long."""
            exp_pr, ix_pr, it = state
            GP = NGP // parts
            RP = PBLK // parts
            EX = exp_pr.rearrange("p b g a -> p (b g) a")     # [128,528,12]
            AP = ix_pr.bitcast(bf16).rearrange("p b g a -> p (b g) a")
            dens = [dnp.tile([128, GP], f32, name=f"den{h}")
                    for h in range(parts)]
            recs = [dnp.tile([128, GP], f32, name=f"rec{h}")
                    for h in range(parts)]

            # Per-macro tail stages (h = macro half of the previous pair).
            # All packed slices start at even element offsets (2x_1p needs
            # 4B alignment on HW).  Chain per half:
            #   e0 -> e1 -> e2 -> x0 -> m(Pool) -> m1(Pool) -> r -> mul
            def sl(T, h, a0, a1):
                return T[:, h * GP:(h + 1) * GP, a0:a1]

            def e0(h):  # 15: pairs [0:6]+[8:14] -> leaves 6,7,14 (6w, DVE)
                nc.vector.tensor_tensor(out=sl(EX, h, 0, 6),
                                        in0=sl(EX, h, 0, 6),
                                        in1=sl(EX, h, 8, 14), op=add)

            def e1(h):  # [0:4] += [4:8]   (4w packed, DVE)
                nc.vector.tensor_tensor(out=sl(EX, h, 0, 4),
                                        in0=sl(EX, h, 0, 4),
                                        in1=sl(EX, h, 4, 8), op=add)

            def e2(h):  # [0:2] += [2:4]   (2w packed, DVE)
                nc.vector.tensor_tensor(out=sl(EX, h, 0, 2),
                                        in0=sl(EX, h, 0, 2),
                                        in1=sl(EX, h, 2, 4), op=add)

            def p0(h):  # ap[0:2] += ap[2:4]   (2w packed, DVE)
                nc.vector.tensor_tensor(out=sl(AP, h, 0, 2),
                                        in0=sl(AP, h, 0, 2),
                                        in1=sl(AP, h, 2, 4), op=add)

            def x0(h):  # ex[0:2] += ap[0:2]   (2w packed, DVE)
                nc.vector.tensor_tensor(out=sl(EX, h, 0, 2),
                                        in0=sl(EX, h, 0, 2),
                                        in1=sl(AP, h, 0, 2), op=add)

            def n0(h):  # ex1 += ex14   (1w, Pool)
                nc.gpsimd.tensor_tensor(out=sl(EX, h, 1, 2),
                                        in0=sl(EX, h, 1, 2),
                                        in1=sl(EX, h, 14, 15), op=add)

            def m(h):   # den = ex0 + ex1   (bf16 -> f32, Pool)
                nc.gpsimd.tensor_tensor(
                    out=dens[h].rearrange("p (g a) -> p g a", a=1),
                    in0=sl(EX, h, 0, 1), in1=sl(EX, h, 1, 2), op=add)

            def m1(h):  # den += 1   (the dropped a=1 column, Pool)
                nc.gpsimd.tensor_scalar_add(dens[h], dens[h], 1.0)

            def r(h):
                nc.vector.reciprocal_approx_fast(out=recs[h], in_=dens[h])

            def mul(h):
                _, xg = loads_of[it]
                og = outp.tile([128, RP, INPUT], f32, name="og")
                nc.gpsimd.tensor_tensor(
                    out=og, in0=xg[:, h * RP:(h + 1) * RP],
                    in1=recs[h].rearrange("p (t f) -> p t f", f=INPUT),
                    op=mult)
                g = iters[it]
                m0 = g * PBLK + h * RP
                nc.sync.dma_start(
                    out=y_r[m0:m0 + RP].rearrange("m p f -> p m f"),
                    in_=og,
                )

            from functools import partial
            out = []
            for h in range(parts):
                for st in (e0, e1, e2, p0, x0, n0, m, m1, r, mul):
                    out.append(partial(st, h))
            return out

        loads_of = {0: emit_loads(0)}
        if len(iters) > 1:
            loads_of[1] = emit_loads(1)
        front_of = {0: emit_front_pair(0)}
        pending = None
        for it in range(len(iters)):
            if it + 2 < len(iters):
                loads_of[it + 2] = emit_loads(it + 2)
            pending = emit_pair(it, pending)
            front_of.pop(it, None)
            if it > 1:
                loads_of.pop(it - 2, None)
        for stage in _tail_stages(pending, parts=4):
            stage()

    nc.compile()
    return nc


def _select_columns(x, E_W, E_b, A_W, A_b):
    """Per-i ranking of the 19 kept softmax columns by mean softmax mass
    (8k-row sample).  Returns [66, 19] array of kept-column indices,
    largest-mass first (first N_EXACT -> exact, rest -> approx)."""
    rng = np.random.default_rng(0)
    n = min(8192, x.shape[0])
    xs = x[:n].astype(np.float32)
    E = np.tanh(xs @ E_W + E_b)
    s = np.einsum("be,iea->bia", E, A_W.astype(np.float32)) + A_b
    sm = s - s.max(axis=2, keepdims=True)
    p = np.exp(sm)
    p /= p.sum(axis=2, keepdims=True)
    mass = p.mean(axis=0)                       # [66, 20]
    keep = np.array([a for a in range(A_NODE) if a != 1])
    mass19 = mass[:, keep]                      # [66, 19]
    order = np.argsort(-mass19, axis=1)         # descending mass
    return keep[order]                          # [66, 19] original indices


def _prep_weights(x, E_W, E_b, A_W, A_b):
    E_W = np.asarray(E_W, dtype=np.float32)
    E_b = np.asarray(E_b, dtype=np.float32)
    A_W = np.asarray(A_W, dtype=np.float32)
    A_b = np.asarray(A_b, dtype=np.float32)
    w1 = np.concatenate([E_W, np.zeros((INPUT, 1), np.float32)], axis=1)
    b1 = np.concatenate([E_b, np.float32([CONST_ROW_BIAS])]).reshape(-1, 1)
    dW = A_W - A_W[:, :, 1:2]                        # [66, 50, 20]
    db = A_b - A_b[:, 1:2]                           # [66, 20]
    cols = _select_columns(x, E_W, E_b, A_W, A_b)    # [66, 19]
    # reorder per-i: exact (large mass) first, then approx
    dW_s = np.take_along_axis(dW, cols[:, None, :], axis=2)   # [66, 50, 19]
    db_s = np.take_along_axis(db, cols, axis=1)               # [66, 19]
    w2f = np.concatenate(
        [dW_s.transpose(1, 0, 2),
         db_s.reshape(INPUT, 1, A_RED).transpose(1, 0, 2)], axis=0,
    ).astype(np.float64)                             # [51, 66, 19]
    # Schraudolph-16 folding for the last N_APPROX cols of each i:
    # t = s*128*log2(e) + 128*(127-c); bias rides the const-1 ET row.
    B16 = (127.0 - SCHRAUD_C) * 128.0
    w2f[:, :, N_EXACT:] *= SCHRAUD_A
    w2f[E_NODE, :, N_EXACT:] += B16
    # exact: 2 chunks of 33 i-groups; approx: one 264-col chunk (i-major)
    w2c = w2f.reshape(E_NODE + 1, 2, IPC, A_RED)
    w2e = np.zeros((E_NODE + 1, 2, CH_EXP), np.float64)
    w2e[:, :, :CH_EX] = w2c[:, :, :, :N_EXACT].reshape(E_NODE + 1, 2, CH_EX)
    w2e = w2e.reshape(E_NODE + 1, 2 * CH_EXP).astype(np.float32)
    w2a = np.ascontiguousarray(
        w2f[:, :, N_EXACT:].reshape(E_NODE + 1, CH_AP).astype(np.float32))
    return np.ascontiguousarray(w1), np.ascontiguousarray(b1), \
        np.ascontiguousarray(w2e), np.ascontiguousarray(w2a)


def _make_in_maps(x, E_W, E_b, A_W, A_b):
    x = np.ascontiguousarray(np.asarray(x, dtype=np.float32))
    n_local = x.shape[0] // N_CORES
    w1, b1, w2e, w2a = _prep_weights(x, E_W, E_b, A_W, A_b)
    in_maps = []
    for i in range(N_CORES):
        xi = x[i * n_local:(i + 1) * n_local]
        in_maps.append({
            "x": xi,
            "xT": np.ascontiguousarray(xi.T),
            "W1": w1, "b1": b1, "W2E": w2e, "W2A": w2a,
        })
    return in_maps, n_local


def _run(x, E_W, E_b, A_W, A_b, trace=False):
    from concourse.bass_utils import run_bass_kernel_spmd

    in_maps, n_local = _make_in_maps(x, E_W, E_b, A_W, A_b)
    key = ("nc", n_local)
    if key not in _CACHE:
        _CACHE[key] = _build_bass(n_local)
    nc = _CACHE[key]
    res = run_bass_kernel_spmd(nc, in_maps, list(range(N_CORES)), trace=trace)
    out = np.concatenate([res.results[i]["y"] for i in range(N_CORES)], axis=0)
    return out, res


def kernel(x, E_W, E_b, A_W, A_b):
    out, _ = _run(x, E_W, E_b, A_W, A_b, trace=False)
    return out
